# revision 1
# baseline (speedup 1.0000x reference)
"""GAT-KH (2-layer, 3-hop, 8-head GAT, N=50k, E=300k/hop) on 8 TRN2 cores.

Distribution: dst-sharded edges, replicated node-side tables.
- Nodes renumbered into 8 padded shards of 6400 (NP=51200). Core c owns dst
  shard c (tiles of 128 dsts, 50 tiles).
- Per (layer,hop): gather table T_k[n] = [hp(256) | alpha_src(8) |
  alpha_dst(8)] bf16, 768B row stride, computed replicated on every core
  from transposed h and host-folded Wcat = [W | W@a_src | W@a_dst].
- Edge phase: per dst tile, edges chunked by 128 (split src<32768 for int16
  dma_gather), gathered rows -> softmax numerators w=exp(leaky(as+ad)+mb)
  -> one-hot P matmuls scatter [w*hp | w] into PSUM -> normalize -> dec_w
  -> leaky -> decay-accumulate.
- Layer end: LayerNorm+residual; layer-1 h shards AllGathered for layer-2
  tables. Output: per-core h shard, host-concatenated.
"""

import numpy as np
import ml_dtypes
from contextlib import ExitStack

N = 50000
E = 300000
HOPS = 3
LAYERS = 2
HEADS = 8
D = 256
DH = 32
NCORES = 8
SHARD = 6250
SHARD_P = 6400
TILES = SHARD_P // 128            # 50
NP = NCORES * SHARD_P             # 51200
NT = NP // 128                    # 400 table chunks
GT = 10                           # table chunks per staging group (50%GT==0)
ROWB = 384                        # table row length in bf16 elems (768B)
TCOLS = 272
LH_SPLIT = 32768
DECAY = [float(np.exp(-0.5 * k)) for k in range(HOPS)]
SLOPE_ACT = 0.01
SLOPE_ATT = 0.2
LN_EPS = 1e-5
NEG_BIAS = -30000.0
TPG = 5                           # dst tiles per gather group (50%TPG==0)
BF16 = ml_dtypes.bfloat16


def _pack_idx16(idx):
    """int16 idx list -> [128, ceil(n/16)] wrapped in 16 partitions, x8."""
    n = len(idx)
    n16 = max(1, (n + 15) // 16)
    a = np.zeros((16, n16), np.int16)
    for p in range(16):
        seg = idx[p::16]
        a[p, : len(seg)] = seg
    return np.tile(a, (8, 1))


def _host_prep(inputs):
    x = np.asarray(inputs["x"], np.float32)
    ei = np.asarray(inputs["edge_index_k_hops"])
    lin1_w = np.asarray(inputs["lin1_w"], np.float32)
    gat_w = np.asarray(inputs["gat_w"], np.float32)
    a_src = np.asarray(inputs["gat_att_src"], np.float32)
    a_dst = np.asarray(inputs["gat_att_dst"], np.float32)
    dec_w = np.asarray(inputs["dec_w"], np.float32)

    wcat = np.zeros((LAYERS, HOPS, D, TCOLS), np.float32)
    for l in range(LAYERS):
        for k in range(HOPS):
            W = gat_w[l, k]
            Wh = W.reshape(D, HEADS, DH)
            wcat[l, k, :, :D] = W
            wcat[l, k, :, D:D + HEADS] = np.einsum("dhc,hc->dh", Wh, a_src[l, k])
            wcat[l, k, :, D + HEADS:] = np.einsum("dhc,hc->dh", Wh, a_dst[l, k])

    xT = np.zeros((D, NP), np.float32)
    for s in range(NCORES):
        xs = x[s * SHARD:(s + 1) * SHARD]
        xT[:, s * SHARD_P: s * SHARD_P + xs.shape[0]] = xs.T
    xT_bf = xT.astype(BF16)
    # per-core shard slice of xT, shaped [2,128,SHARD_P]
    xTs = [
        xT_bf[:, c * SHARD_P:(c + 1) * SHARD_P].reshape(2, 128, SHARD_P)
        for c in range(NCORES)
    ]

    # ---- edge routing ----
    hopdat = []
    cnts = np.zeros((HOPS, NCORES, TILES, 2), np.int64)
    for k in range(HOPS):
        src = ei[k, 0].astype(np.int64)
        dst = ei[k, 1].astype(np.int64)
        ps = (src // SHARD) * SHARD_P + (src % SHARD)
        core = dst // SHARD
        dl = dst % SHARD
        tl = dl // 128
        dloc = dl % 128
        low = ps < LH_SPLIT
        hopdat.append((ps, core, tl, dloc, low))
        for c in range(NCORES):
            m = core == c
            tls = tl[m]
            lows = low[m]
            for t in range(TILES):
                mt = tls == t
                cnts[k, c, t, 0] = int((mt & lows).sum())
                cnts[k, c, t, 1] = int((mt & ~lows).sum())
    nch = np.maximum(1, np.ceil(cnts.max(axis=1) / 128.0)).astype(np.int64)  # [HOPS,TILES,2]

    core_data = []
    for c in range(NCORES):
        hops = []
        for k in range(HOPS):
            ps, core, tl, dloc, low = hopdat[k]
            m = core == c
            Lparts, Hparts, dcols, mcols = [], [], [], []
            for t in range(TILES):
                mt = m & (tl == t)
                for side in (0, 1):
                    msk = mt & (low if side == 0 else ~low)
                    n_real = int(msk.sum())
                    cap = int(nch[k, t, side]) * 128
                    idx = np.zeros(cap, np.int64)
                    dv = np.zeros(cap, np.int64)
                    mb = np.full(cap, NEG_BIAS, np.float32)
                    idx[:n_real] = ps[msk] - (LH_SPLIT if side else 0)
                    dv[:n_real] = dloc[msk]
                    mb[:n_real] = 0.0
                    (Lparts if side == 0 else Hparts).append(idx)
                    nc_ = cap // 128
                    dcols.append(dv.reshape(nc_, 128).T.astype(np.float32))
                    mcols.append(mb.reshape(nc_, 128).T.astype(np.float32))
            Lidx = np.concatenate(Lparts).astype(np.int16)
            Hidx = np.concatenate(Hparts).astype(np.int16)
            dstloc = np.concatenate(dcols, axis=1)
            # flat edge-order dst-local row: dstrow[chunk*128+p] = dstloc[p, chunk]
            dstrow = dstloc.T.reshape(1, -1).astype(BF16)
            hops.append({
                "Lidx": _pack_idx16(Lidx), "Hidx": _pack_idx16(Hidx),
                "dstloc": dstloc, "dstrow": dstrow,
                "maskb": np.concatenate(mcols, axis=1),
            })
        core_data.append(hops)

    iota = np.tile(np.arange(128, dtype=np.float32)[None, :], (128, 1))
    iotac = np.ascontiguousarray(np.arange(128, dtype=np.float32)[:, None])
    return {
        "iotac": iotac,
        "wcat": wcat.astype(BF16), "xT": xT_bf, "xTs": xTs,
        "lin1": lin1_w.astype(BF16), "dec": dec_w.astype(BF16),
        "core_data": core_data, "nch": nch, "iota": iota,
    }


def _build(prep, stage=5, edgelvl=5):
    from concourse import bass, mybir, tile, library_config
    from concourse.bass import AP
    from concourse.masks import make_identity
    import concourse.bacc as bacc

    nch = prep["nch"]
    cd0 = prep["core_data"][0]
    NCH = [cd0[k]["dstloc"].shape[1] for k in range(HOPS)]
    NIDXL = [cd0[k]["Lidx"].shape[1] * 16 for k in range(HOPS)]
    NIDXH = [cd0[k]["Hidx"].shape[1] * 16 for k in range(HOPS)]

    fp32 = mybir.dt.float32
    bf16 = mybir.dt.bfloat16
    i16 = mybir.dt.int16
    AF = mybir.ActivationFunctionType
    OP = mybir.AluOpType

    nc = bacc.Bacc("TRN2", target_bir_lowering=False, debug=False,
                   num_devices=NCORES)

    d_xT = nc.dram_tensor("xT", [D, NP], bf16, kind="ExternalInput")
    d_xTs = nc.dram_tensor("xTs", [2, 128, SHARD_P], bf16, kind="ExternalInput")
    d_lin1 = nc.dram_tensor("lin1", [D, D], bf16, kind="ExternalInput")
    d_wcat = nc.dram_tensor("wcat", [LAYERS, HOPS, D, TCOLS], bf16, kind="ExternalInput")
    d_dec = nc.dram_tensor("dec", [LAYERS, HOPS, D, D], bf16, kind="ExternalInput")
    d_iota = nc.dram_tensor("iota", [128, 128], fp32, kind="ExternalInput")
    d_gidx = [(nc.dram_tensor(f"gidxL{k}", [128, NIDXL[k] // 16], i16, kind="ExternalInput"),
               nc.dram_tensor(f"gidxH{k}", [128, NIDXH[k] // 16], i16, kind="ExternalInput"))
              for k in range(HOPS)]
    d_dstloc = [nc.dram_tensor(f"dstloc{k}", [128, NCH[k]], fp32, kind="ExternalInput") for k in range(HOPS)]
    d_dstrow = [nc.dram_tensor(f"dstrow{k}", [1, NCH[k] * 128], bf16, kind="ExternalInput") for k in range(HOPS)]
    d_iotac = nc.dram_tensor("iotac", [128, 1], fp32, kind="ExternalInput")
    d_maskb = [nc.dram_tensor(f"maskb{k}", [128, NCH[k]], fp32, kind="ExternalInput") for k in range(HOPS)]
    d_out = nc.dram_tensor("out", [SHARD_P, D], fp32, kind="ExternalOutput")

    d_tab = [nc.dram_tensor(f"tab{k}", [NP, ROWB], bf16, kind="Internal") for k in range(HOPS)]
    d_hT0 = nc.dram_tensor("hT0", [2, 128, NP], bf16, kind="Internal")
    d_agin = nc.dram_tensor("agin", [2, 128, SHARD_P], bf16, kind="Internal")
    d_agout = nc.dram_tensor("agout", [NCORES, 2, 128, SHARD_P], bf16,
                             kind="Internal", addr_space="Shared")

    # per-hop static chunk bookkeeping
    # chunk columns: per tile t: L chunks then H chunks
    col0 = []   # [k][t] first chunk col of tile t
    lcum = []   # [k][t] first L-chunk index (global, within hop) of tile t
    hcum = []
    for k in range(HOPS):
        c0, lc, hc = [], [], []
        a = b = g = 0
        for t in range(TILES):
            c0.append(g)
            lc.append(a)
            hc.append(b)
            a += int(nch[k, t, 0])
            b += int(nch[k, t, 1])
            g += int(nch[k, t, 0] + nch[k, t, 1])
        col0.append(c0)
        lcum.append(lc)
        hcum.append(hc)

    groups = [list(range(g0, g0 + TPG)) for g0 in range(0, TILES, TPG)]

    def bcast_mid(ap, n):
        """[128, M] AP -> [128, n, M] with 0-step middle dim."""
        return AP(ap.tensor, ap.offset, [list(ap.ap[0]), [0, n], list(ap.ap[1])])

    with tile.TileContext(nc) as tc:
        with ExitStack() as ctx:
            persist = ctx.enter_context(tc.tile_pool(name="persist", bufs=1))
            nc.gpsimd.load_library(library_config.mlp)

            sb_iota = persist.tile((128, 128), fp32)
            nc.sync.dma_start(sb_iota[:], d_iota[:, :])
            sb_ident = persist.tile((128, 128), bf16)
            make_identity(nc, sb_ident[:])
            sb_lin1 = persist.tile((128, 2, D), bf16)
            for kc in range(2):
                nc.sync.dma_start(sb_lin1[:, kc, :], d_lin1[kc * 128:(kc + 1) * 128, :])
            sb_wcat = persist.tile((128, LAYERS, HOPS, 2, TCOLS), bf16)
            sb_dec = persist.tile((128, LAYERS, HOPS, 2, D), bf16)
            for l in range(LAYERS):
                for k in range(HOPS):
                    for kc in range(2):
                        nc.sync.dma_start(sb_wcat[:, l, k, kc, :],
                                          d_wcat[l, k, kc * 128:(kc + 1) * 128, :])
                        nc.sync.dma_start(sb_dec[:, l, k, kc, :],
                                          d_dec[l, k, kc * 128:(kc + 1) * 128, :])
            sb_acc = persist.tile((128, TILES, D), fp32)
            sb_res = persist.tile((128, TILES, D), bf16)
            sb_eps = persist.tile((128, 1), fp32)
            nc.vector.memset(sb_eps[:], LN_EPS)
            sb_iotac = persist.tile((128, 1), fp32)
            nc.sync.dma_start(sb_iotac[:], d_iotac[:, :])
            sb_ad = persist.tile((128, TILES, HOPS, 8), bf16)

            # ---------- phase 0a: h0T full (replicated) ----------
            SUP = 512
            with tc.tile_pool(name="p0", bufs=3) as p0, \
                 tc.tile_pool(name="p0ps", bufs=4, space="PSUM") as p0ps:
                for st in range(NP // SUP):
                    xt = p0.tile((128, 2, SUP), bf16, tag="xt")
                    for kc in range(2):
                        nc.sync.dma_start(xt[:, kc, :],
                                          d_xT[kc * 128:(kc + 1) * 128, st * SUP:(st + 1) * SUP])
                    for mc in range(2):
                        ps = p0ps.tile((128, SUP), fp32, tag="ps")
                        for kc in range(2):
                            nc.tensor.matmul(ps[:], lhsT=sb_lin1[:, kc, mc * 128:(mc + 1) * 128],
                                             rhs=xt[:, kc, :], start=(kc == 0), stop=(kc == 1))
                        r = p0.tile((128, SUP), fp32, tag="r")
                        nc.scalar.activation(r[:], ps[:], AF.Relu, scale=1.0 - SLOPE_ACT)
                        h0 = p0.tile((128, SUP), bf16, tag="h0")
                        nc.vector.tensor_scalar(h0[:], ps[:], SLOPE_ACT, None, OP.mult)
                        nc.vector.tensor_add(h0[:], h0[:], r[:])
                        nc.sync.dma_start(d_hT0[mc, :, st * SUP:(st + 1) * SUP], h0[:])

            # ---------- phase 0b: residual h0 for own shard ----------
            with tc.tile_pool(name="p0b", bufs=3) as p0b, \
                 tc.tile_pool(name="p0bps", bufs=2, space="PSUM") as p0bps:
                for t in range(TILES):
                    xs = p0b.tile((128, 2, 128), bf16, tag="xs")
                    for kc in range(2):
                        nc.sync.dma_start(xs[:, kc, :], d_xTs[kc, :, t * 128:(t + 1) * 128])
                    ps = p0bps.tile((128, D), fp32, tag="ps")
                    for kc in range(2):
                        nc.tensor.matmul(ps[:], lhsT=xs[:, kc, :], rhs=sb_lin1[:, kc, :],
                                         start=(kc == 0), stop=(kc == 1))
                    r = p0b.tile((128, D), fp32, tag="r")
                    nc.scalar.activation(r[:], ps[:], AF.Relu, scale=1.0 - SLOPE_ACT)
                    t1 = p0b.tile((128, D), fp32, tag="t1")
                    nc.vector.tensor_scalar(t1[:], ps[:], SLOPE_ACT, None, OP.mult)
                    nc.vector.tensor_add(sb_res[:, t, :], t1[:], r[:])

            # ================= layers =================
            for l in range(LAYERS if stage >= 5 else 1):
                # ---- alpha_dst per own-shard tile ----
                with tc.tile_pool(name=f"ad{l}", bufs=3) as pad, \
                     tc.tile_pool(name=f"adps{l}", bufs=4, space="PSUM") as padps:
                    for t in range(TILES):
                        hts = pad.tile((128, 2, 128), bf16, tag="hts")
                        if l == 0:
                            xs2 = pad.tile((128, 2, 128), bf16, tag="xs2")
                            for kc in range(2):
                                nc.sync.dma_start(xs2[:, kc, :], d_xTs[kc, :, t * 128:(t + 1) * 128])
                            for mc in range(2):
                                pst = padps.tile((128, 128), fp32, tag="pst")
                                for kc in range(2):
                                    nc.tensor.matmul(pst[:], lhsT=sb_lin1[:, kc, mc * 128:(mc + 1) * 128],
                                                     rhs=xs2[:, kc, :], start=(kc == 0), stop=(kc == 1))
                                rr = pad.tile((128, 128), fp32, tag="rr")
                                nc.scalar.activation(rr[:], pst[:], AF.Relu, scale=1.0 - SLOPE_ACT)
                                tt = pad.tile((128, 128), fp32, tag="tt")
                                nc.vector.tensor_scalar(tt[:], pst[:], SLOPE_ACT, None, OP.mult)
                                nc.vector.tensor_add(hts[:, mc, :], tt[:], rr[:])
                        else:
                            for kc in range(2):
                                nc.sync.dma_start(hts[:, kc, :], d_agin[kc, :, t * 128:(t + 1) * 128])
                        psa = padps.tile((128, HOPS * 8), fp32, tag="psa")
                        for k in range(HOPS):
                            for kc in range(2):
                                nc.tensor.matmul(psa[:, k * 8:(k + 1) * 8],
                                                 lhsT=hts[:, kc, :],
                                                 rhs=sb_wcat[:, l, k, kc, D + HEADS:D + 2 * HEADS],
                                                 start=(kc == 0), stop=(kc == 1),
                                                 skip_group_check=True)
                        nc.vector.tensor_copy(
                            sb_ad[:, t, :, :].rearrange("p k h -> p (k h)"), psa[:])

                # ---- tables for all 3 hops ----
                if stage < 1:
                    break
                with tc.tile_pool(name=f"tb{l}", bufs=2) as ptb, \
                     tc.tile_pool(name=f"tbps{l}", bufs=2, space="PSUM") as ptbps:
                    for g in range(NT // GT):
                        n0 = g * GT * 128
                        hTc = ptb.tile((128, 2, GT, 128), bf16, tag="hTc")
                        for kc in range(2):
                            if l == 0:
                                nc.sync.dma_start(
                                    hTc[:, kc, :, :].rearrange("p g c -> p (g c)"),
                                    d_hT0[kc, :, n0:n0 + GT * 128])
                            else:
                                s = (g * GT) // TILES
                                j0 = (g * GT) % TILES
                                nc.sync.dma_start(
                                    hTc[:, kc, :, :].rearrange("p g c -> p (g c)"),
                                    d_agout[s, kc, :, j0 * 128:j0 * 128 + GT * 128])
                        stg = [ptb.tile((128, GT, TCOLS), bf16, tag=f"st{k}", name=f"stg{l}_{g}_{k}") for k in range(HOPS)]
                        for ci in range(GT):
                            pks = [ptbps.tile((128, TCOLS), fp32, tag=f"tp{k}", name=f"pks{l}_{g}_{ci}_{k}") for k in range(HOPS)]
                            for kc in range(2):
                                for k in range(HOPS):
                                    nc.tensor.matmul(pks[k][:], lhsT=hTc[:, kc, ci, :],
                                                     rhs=sb_wcat[:, l, k, kc, :],
                                                     start=(kc == 0), stop=(kc == 1),
                                                     skip_group_check=True)
                            for k in range(HOPS):
                                if k == 1:
                                    nc.scalar.copy(stg[k][:, ci, :], pks[k][:])
                                else:
                                    nc.vector.tensor_copy(stg[k][:, ci, :], pks[k][:])
                        for k in range(HOPS):
                            nc.sync.dma_start(
                                d_tab[k][n0:n0 + GT * 128, 0:TCOLS]
                                .rearrange("(g p) c -> p g c", p=128),
                                stg[k][:])

                # ---- edge phase: 3 hops ----
                nhops = HOPS if stage >= 3 else (1 if stage >= 2 else 0)
                for k in range(nhops):
                    dl_k = d_gidx[k]
                    with tc.tile_pool(name=f"eg{l}{k}", bufs=2) as peg, \
                         tc.tile_pool(name=f"egsc{l}{k}", bufs=2, space="PSUM") as psc, \
                         tc.tile_pool(name=f"egtr{l}{k}", bufs=2, space="PSUM") as ptr, \
                         tc.tile_pool(name=f"egdc{l}{k}", bufs=2, space="PSUM") as pdc:
                        for grp in groups:
                            t0 = grp[0]
                            gnL = int(sum(nch[k, t, 0] for t in grp))
                            gnH = int(sum(nch[k, t, 1] for t in grp))
                            gnc = int(sum(nch[k, t, 0] + nch[k, t, 1] for t in grp))
                            oL = lcum[k][t0]
                            oH = hcum[k][t0]
                            oc = col0[k][t0]

                            GSL = 28  # chunks per dma_gather
                            ixL = peg.tile((128, gnL * 8), i16, tag="ixL")
                            nc.sync.dma_start(ixL[:], dl_k[0][:, oL * 8:(oL + gnL) * 8])
                            gbL = peg.tile((128, gnL, ROWB), bf16, tag="gbL")
                            for off in range(0, gnL, GSL):
                                cnt = min(GSL, gnL - off)
                                nc.gpsimd.dma_gather(
                                    out_ap=gbL[:, off:off + cnt, :],
                                    in_ap=d_tab[k][0:LH_SPLIT, :],
                                    idxs_ap=ixL[:, off * 8:(off + cnt) * 8],
                                    num_idxs=cnt * 128,
                                    num_idxs_reg=cnt * 128, elem_size=ROWB,
                                    single_packet=False)
                            ixH = peg.tile((128, gnH * 8), i16, tag="ixH")
                            nc.sync.dma_start(ixH[:], dl_k[1][:, oH * 8:(oH + gnH) * 8])
                            gbH = peg.tile((128, gnH, ROWB), bf16, tag="gbH")
                            for off in range(0, gnH, GSL):
                                cnt = min(GSL, gnH - off)
                                nc.gpsimd.dma_gather(
                                    out_ap=gbH[:, off:off + cnt, :],
                                    in_ap=d_tab[k][LH_SPLIT:NP, :],
                                    idxs_ap=ixH[:, off * 8:(off + cnt) * 8],
                                    num_idxs=cnt * 128,
                                    num_idxs_reg=cnt * 128, elem_size=ROWB,
                                    single_packet=False)
                            dls = peg.tile((128, gnc), fp32, tag="dls")
                            nc.sync.dma_start(dls[:], d_dstloc[k][:, oc:oc + gnc])
                            mbs = peg.tile((128, gnc), fp32, tag="mbs")
                            nc.sync.dma_start(mbs[:], d_maskb[k][:, oc:oc + gnc])
                            drow = peg.tile((128, gnc * 128), bf16, tag="drow")
                            nc.sync.dma_start(drow[0:1, :], d_dstrow[k][0:1, oc * 128:(oc + gnc) * 128])
                            nc.gpsimd.partition_broadcast(drow[:], drow[0:1, :])

                            for t in grp:
                                if edgelvl < 2:
                                    continue
                                nL = int(nch[k, t, 0])
                                nH = int(nch[k, t, 1])
                                nT = nL + nH
                                lj = lcum[k][t] - oL
                                hj = hcum[k][t] - oH
                                cj = col0[k][t] - oc

                                V = peg.tile((128, nT, 264), bf16, tag="V")
                                e0 = peg.tile((128, nT, 8), fp32, tag="e0")
                                e1 = peg.tile((128, nT, 8), fp32, tag="e1")
                                PT = peg.tile((128, nT * 128), bf16, tag="PT")
                                nc.vector.tensor_scalar(
                                    PT[:], drow[:, cj * 128:(cj + nT) * 128],
                                    sb_iotac[:, 0:1], None, OP.is_equal)
                                pad_ps = psc.tile((128, nT, 8), fp32, tag="pad_ps")
                                for j in range(nT):
                                    nc.tensor.matmul(pad_ps[:, j, :],
                                                     lhsT=PT[:, j * 128:(j + 1) * 128],
                                                     rhs=sb_ad[:, t, k, :],
                                                     start=True, stop=True,
                                                     skip_group_check=True)
                                nc.vector.tensor_tensor(
                                    e0[:, 0:nL, :], gbL[:, lj:lj + nL, 256:264],
                                    pad_ps[:, 0:nL, :], op=OP.add)
                                nc.vector.tensor_tensor(
                                    e0[:, nL:nT, :], gbH[:, hj:hj + nH, 256:264],
                                    pad_ps[:, nL:nT, :], op=OP.add)
                                nc.vector.tensor_scalar(e1[:], e0[:], SLOPE_ATT, None, OP.mult)
                                nc.vector.tensor_tensor(e1[:], e0[:], e1[:], op=OP.max)
                                nc.vector.tensor_tensor(
                                    e1[:], e1[:],
                                    mbs[:, cj:cj + nT].to_broadcast((128, nT, 8)),
                                    op=OP.add)
                                nc.scalar.activation(V[:, :, 256:264], e1[:], AF.Exp)
                                if edgelvl < 3:
                                    continue
                                for j in range(nT):
                                    src = (gbL[:, lj + j, 0:256] if j < nL
                                           else gbH[:, hj + (j - nL), 0:256])
                                    nc.vector.tensor_tensor(
                                        V[:, j, 0:256].rearrange("p (h c) -> p h c", h=HEADS),
                                        src.rearrange("p (h c) -> p h c", h=HEADS),
                                        V[:, j, 256:264].to_broadcast((128, HEADS, DH)),
                                        op=OP.mult)
                                if edgelvl < 4:
                                    continue
                                P = peg.tile((128, nT, 128), bf16, tag="P")
                                nc.vector.tensor_tensor(
                                    P[:], dls[:, cj:cj + nT].to_broadcast((128, nT, 128)),
                                    bcast_mid(sb_iota[:], nT), op=OP.is_equal)
                                ps = psc.tile((128, 264), fp32, tag="ps")
                                for j in range(nT):
                                    nc.tensor.matmul(ps[:], lhsT=P[:, j, :], rhs=V[:, j, :],
                                                     start=(j == 0), stop=(j == nT - 1))
                                den = peg.tile((128, 8), fp32, tag="den")
                                nc.vector.tensor_scalar(den[:], ps[:, 256:264], 1e-16, None, OP.add)
                                rec = peg.tile((128, 8), fp32, tag="rec")
                                nc.vector.reciprocal(rec[:], den[:])
                                Gt = peg.tile((128, D), bf16, tag="Gt")
                                nc.vector.tensor_tensor(
                                    Gt[:].rearrange("p (h c) -> p h c", h=HEADS),
                                    ps[:, 0:256].rearrange("p (h c) -> p h c", h=HEADS),
                                    rec[:].to_broadcast((128, HEADS, DH)), op=OP.mult)
                                if edgelvl < 5:
                                    continue
                                GtT = peg.tile((128, 2, 128), bf16, tag="GtT")
                                for kc in range(2):
                                    pt = ptr.tile((128, 128), bf16, tag="pt")
                                    nc.tensor.transpose(pt[:], Gt[:, kc * 128:(kc + 1) * 128],
                                                        sb_ident[:])
                                    nc.vector.tensor_copy(GtT[:, kc, :], pt[:])
                                pd = pdc.tile((128, D), fp32, tag="pd")
                                for kc in range(2):
                                    nc.tensor.matmul(pd[:], lhsT=GtT[:, kc, :],
                                                     rhs=sb_dec[:, l, k, kc, :],
                                                     start=(kc == 0), stop=(kc == 1))
                                u1 = peg.tile((128, D), fp32, tag="u1")
                                nc.vector.tensor_scalar(u1[:], pd[:], DECAY[k], None, OP.mult)
                                u2 = peg.tile((128, D), fp32, tag="u2")
                                nc.scalar.activation(u2[:], pd[:], AF.Copy,
                                                     scale=DECAY[k] * SLOPE_ACT)
                                if k == 0:
                                    nc.vector.tensor_tensor(sb_acc[:, t, :], u1[:], u2[:], op=OP.max)
                                else:
                                    u3 = peg.tile((128, D), fp32, tag="u3")
                                    nc.vector.tensor_tensor(u3[:], u1[:], u2[:], op=OP.max)
                                    nc.vector.tensor_add(sb_acc[:, t, :], sb_acc[:, t, :], u3[:])

                # ---- layer norm + residual (+ agin / output) ----
                if stage < 3 or edgelvl < 5:
                    break
                with tc.tile_pool(name=f"ln{l}", bufs=3) as pln, \
                     tc.tile_pool(name=f"lnps{l}", bufs=2, space="PSUM") as plnps:
                    for t in range(TILES):
                        acc = sb_acc[:, t, :]
                        mu = pln.tile((128, 1), fp32, tag="mu")
                        nc.vector.reduce_sum(mu[:], acc, axis=mybir.AxisListType.X)
                        nc.vector.tensor_scalar(mu[:], mu[:], 1.0 / D, None, OP.mult)
                        xc = pln.tile((128, D), fp32, tag="xc")
                        nc.vector.tensor_scalar(xc[:], acc, mu[:, 0:1], None, OP.subtract)
                        sq = pln.tile((128, D), fp32, tag="sq")
                        nc.vector.tensor_tensor(sq[:], xc[:], xc[:], op=OP.mult)
                        var = pln.tile((128, 1), fp32, tag="var")
                        nc.vector.reduce_sum(var[:], sq[:], axis=mybir.AxisListType.X)
                        sd = pln.tile((128, 1), fp32, tag="sd")
                        nc.scalar.activation(sd[:], var[:], AF.Sqrt, bias=sb_eps[:], scale=1.0 / D)
                        nc.vector.reciprocal(sd[:], sd[:])
                        hn = pln.tile((128, D), fp32, tag="hn")
                        nc.vector.tensor_tensor(hn[:], xc[:], sd[:].to_broadcast((128, D)),
                                                op=OP.mult)
                        nc.vector.tensor_add(hn[:], hn[:], sb_res[:, t, :])
                        if l == 0:
                            nc.vector.tensor_copy(sb_res[:, t, :], hn[:])
                            for kc in range(2):
                                pt = plnps.tile((128, 128), bf16, tag="pt")
                                nc.tensor.transpose(pt[:], sb_res[:, t, kc * 128:(kc + 1) * 128],
                                                    sb_ident[:])
                                hb = pln.tile((128, 128), bf16, tag="hb")
                                nc.vector.tensor_copy(hb[:], pt[:])
                                nc.sync.dma_start(d_agin[kc, :, t * 128:(t + 1) * 128], hb[:])
                        else:
                            nc.sync.dma_start(d_out[t * 128:(t + 1) * 128, :], hn[:])

                if l == 0 and stage >= 4:
                    nc.gpsimd.collective_compute(
                        "AllGather", mybir.AluOpType.bypass,
                        replica_groups=[list(range(NCORES))],
                        ins=[d_agin[:, :, :]],
                        outs=[d_agout[:, :, :, :]],
                    )

            if stage < 5:
                with tc.tile_pool(name="dbg", bufs=2) as pdbg:
                    for t in range(TILES):
                        z = pdbg.tile((128, D), fp32, tag="z")
                        nc.vector.tensor_copy(z[:], sb_res[:, t, :])
                        nc.sync.dma_start(d_out[t * 128:(t + 1) * 128, :], z[:])

    nc.compile()
    return nc


def kernel(**inputs):
    import os
    from concourse.bass_utils import run_bass_kernel_spmd

    prep = _host_prep(inputs)
    nc = _build(prep, stage=int(os.environ.get("GAT_STAGE", "5")), edgelvl=int(os.environ.get("GAT_EDGELVL", "5")))

    in_maps = []
    for c in range(NCORES):
        m = {
            "xT": prep["xT"], "xTs": np.ascontiguousarray(prep["xTs"][c]),
            "lin1": prep["lin1"], "wcat": prep["wcat"], "dec": prep["dec"],
            "iota": prep["iota"], "iotac": prep["iotac"],
        }
        for k in range(HOPS):
            cd = prep["core_data"][c][k]
            m[f"gidxL{k}"] = cd["Lidx"]
            m[f"gidxH{k}"] = cd["Hidx"]
            m[f"dstloc{k}"] = cd["dstloc"]
            m[f"dstrow{k}"] = cd["dstrow"]
            m[f"maskb{k}"] = cd["maskb"]
        in_maps.append(m)

    res = run_bass_kernel_spmd(nc, in_maps, core_ids=list(range(NCORES)))
    kernel.last_exec_ns = res.exec_time_ns

    out = np.zeros((N, D), np.float32)
    for c in range(NCORES):
        out[c * SHARD:(c + 1) * SHARD] = res.results[c]["out"][:SHARD]
    return out

